# revision 18
# baseline (speedup 1.0000x reference)
"""LinOSS layer Trainium2 kernel, v4.2.

Math (same closed form as v3): the per-state 2x2 recurrence has eigenvalues
e^{+-i theta}; the scanned state collapses to rank-2 modulated prefix sums

    u     = s * Bu                     (s folded into B on host)
    E     = cumsum(T1 * u);  F = cumsum(T2 * u)     per complex part
    x_t   = sin(t th) * E_t + cos(t th) * F_t
    T1    = gamma*cos + sin;  T2 = cos - gamma*sin

Structure (keeps the 128 = 4 time-chunks x 32 states partition fold):
  - EVEN/ODD TIME SPLIT, done on the host: inpT columns are permuted per
    2048-chunk to [evens 1024 | odds 1024]; all tables pre-blocked to match.
    The DVE scan (2 cycles/col, no perf modes) then only runs over the
    1024 pair-sums P_j = y_{2j} + y_{2j+1}:
        S_{2j+1} = seed + cumsum(P)_j          (scan, half length)
        S_{2j}   = S_{2j+1} - y_{2j+1}         (aligned 2x tensor_tensor sub)
    halving the dominant scan cost.
  - u is evacuated from PSUM to bf16 SBUF by the ACT engine (4 separate
    e/o PSUM tiles so the even evacs start mid-load), so modulations and
    demodulations are all-bf16 full-width [128,2048] tensor_tensor ops in
    DVE 2x mode.
  - per-chunk carry sums come from ACT activation(Identity, accum_out=..)
    re-reading the pair-sums (off the DVE critical path); Wm matmul turns
    them into scan initial values (v3 mechanism).
  - DMA issue cost is ~0.6us PER dma_start on a HWDGE ring (measured): the
    issue stream is split across BOTH rings (sync + scalar), small tensors
    are packed into one transfer, and out-DMAs batch 2 evacs each.
  - projection/output as v3: two slabs out0/out1, host sums + un-permutes.
"""

import numpy as np

L, H, P = 8192, 128, 256
NCORES = 8
SLOC = P // NCORES          # states per core
FOLD = 4                    # time chunks folded into partitions
CL = L // FOLD              # 2048 free columns per partition row
HCL = CL // 2               # 1024 columns per even/odd half
NPART = FOLD * SLOC         # 128
JT = 512                    # matmul j-tile width

_CACHE: dict = {}


def _build_bass(split_waits=True):
    import concourse.bass as bass
    import concourse.mybir as mybir
    import concourse.tile as tile

    dt = mybir.dt.float32
    bt = mybir.dt.bfloat16
    Alu = mybir.AluOpType
    AF = mybir.ActivationFunctionType

    nc = bass.Bass(
        trn_type="TRN2",
        target_bir_lowering=False,
        debug=False,
        num_devices=NCORES,
    )

    inpT_d = nc.dram_tensor("inpT", [H, L], bt, kind="ExternalInput").ap()
    # packed: Bt [*,0:64] | Ctr [*,64:192] | Cti [*,192:320]
    BCC_d = nc.dram_tensor("BCC", [128, 320], bt, kind="ExternalInput").ap()
    Wm_d = nc.dram_tensor("Wm", [NPART, NPART], dt, kind="ExternalInput").ap()
    T1_d = nc.dram_tensor("T1blk", [NPART, CL], bt, kind="ExternalInput").ap()
    T2_d = nc.dram_tensor("T2blk", [NPART, CL], bt, kind="ExternalInput").ap()
    sin_d = nc.dram_tensor("sinblk", [NPART, CL], bt, kind="ExternalInput").ap()
    cos_d = nc.dram_tensor("cosblk", [NPART, CL], bt, kind="ExternalInput").ap()
    out0 = nc.dram_tensor("out0", [H, L], bt, kind="ExternalOutput").ap()
    out1 = nc.dram_tensor("out1", [H, L], bt, kind="ExternalOutput").ap()

    with tile.TileContext(nc) as tc:
        cpool = tc.alloc_tile_pool(name="const", bufs=1)
        big = tc.alloc_tile_pool(name="big", bufs=1)
        stage = tc.alloc_tile_pool(name="stage", bufs=4)
        pbu_re_pool = tc.alloc_tile_pool(name="pbu_re", bufs=1, space="PSUM")
        pbu_ro_pool = tc.alloc_tile_pool(name="pbu_ro", bufs=1, space="PSUM")
        pbu_ie_pool = tc.alloc_tile_pool(name="pbu_ie", bufs=1, space="PSUM")
        pbu_io_pool = tc.alloc_tile_pool(name="pbu_io", bufs=1, space="PSUM")

        # ---- loads; issue cost is ~0.6us per dma_start per ring, so the
        # stream is split across the two HWDGE rings (sync + scalar) ----
        # sync ring: the 8 inpT chunks (evens first), then T2/cos tables.
        inpT = big.tile([128, L], bt, tag="inpT")
        KB = L // 8  # 1024-col dma blocks; evens are dram blocks 0,2,4,6
        for k8 in (0, 2, 4, 6, 1, 3, 5, 7):
            nc.sync.dma_start(
                out=inpT[:, k8 * KB:(k8 + 1) * KB],
                in_=inpT_d[:, k8 * KB:(k8 + 1) * KB],
            )
        # tables are WAR-gated on inpT progress (gpsimd dummy reading an
        # inpT block + writing a dummy slot of the table tile) so the
        # fair-shared DMA bandwidth stays on the critical input chunks
        def gate(name, gate_k8, col=0):
            # the dummy write must overlap the region the gated DMA writes
            dummy = big.tile([NPART, CL], bt, tag=name)
            gd = cpool.tile([1, 8], dt, tag=f"g_{name}_{col}")
            nc.gpsimd.memset(dummy[0:1, col:col + 8], 0.0)
            nc.gpsimd.tensor_tensor(
                gd[:], dummy[0:1, col:col + 8],
                inpT[0:1, gate_k8 * KB:gate_k8 * KB + 8], Alu.add)
            real = big.tile([NPART, CL], bt, tag=name)
            return real

        # T tables split into half-DMAs with staggered gates: the even
        # halves land early enough for the even-side mods to run inside
        # the odd-chunk DMA window, without starving the input stream.
        # Both gate dummies are created BEFORE the final tile object so
        # every read and both DMAs go through the last allocation.
        def gate2(name, k8_e, k8_o):
            gate(name, k8_e, 0)
            return gate(name, k8_o, HCL)

        T2blk = gate2("T2blk", 4, 3)   # evens after 3rd even, odds after 2nd odd
        nc.sync.dma_start(out=T2blk[:, 0:HCL], in_=T2_d[:, 0:HCL])
        nc.sync.dma_start(out=T2blk[:, HCL:CL], in_=T2_d[:, HCL:CL])
        # scalar ring: small packed tensors + T1/sin/cos tables.
        BCC = cpool.tile([128, 320], bt)
        nc.scalar.dma_start(out=BCC[:], in_=BCC_d)
        Wm = cpool.tile([NPART, NPART], dt)
        nc.scalar.dma_start(out=Wm[:], in_=Wm_d)
        T1blk = gate2("T1blk", 2, 1)   # evens after 2nd even, odds after 1st odd
        nc.scalar.dma_start(out=T1blk[:, 0:HCL], in_=T1_d[:, 0:HCL])
        sinblk = gate("sinblk", 3)     # after 2nd odd block
        cosblk = gate("cosblk", 3)
        Ctr = BCC[:, 2 * SLOC:2 * SLOC + H]
        Cti = BCC[:, 2 * SLOC + H:2 * SLOC + 2 * H]

        ones = cpool.tile([NPART, HCL], bt)
        nc.vector.memset(ones[:], 1.0)

        # ---- Bu matmuls; four separate psum tiles so the even-half evacs
        # can start while the odd chunks are still streaming ----
        pbu_re = pbu_re_pool.tile([NPART, HCL], dt, tag="bu_re")
        pbu_ro = pbu_ro_pool.tile([NPART, HCL], dt, tag="bu_ro")
        pbu_ie = pbu_ie_pool.tile([NPART, HCL], dt, tag="bu_ie")
        pbu_io = pbu_io_pool.tile([NPART, HCL], dt, tag="bu_io")
        bs_r = slice(0, SLOC)
        bs_i = slice(SLOC, 2 * SLOC)
        for half, (pr, pi) in ((0, (pbu_re, pbu_ie)), (1, (pbu_ro, pbu_io))):
            for c in range(FOLD):
                ps = slice(c * SLOC, (c + 1) * SLOC)
                for jh in range(2):
                    rhs = inpT[:, c * CL + half * HCL + jh * JT:
                               c * CL + half * HCL + (jh + 1) * JT]
                    js = slice(jh * JT, (jh + 1) * JT)
                    nc.tensor.matmul(pr[ps, js], BCC[:, bs_r], rhs,
                                     start=True, stop=True,
                                     tile_position=(0, c * SLOC))
                    nc.tensor.matmul(pi[ps, js], BCC[:, bs_i], rhs,
                                     start=True, stop=True,
                                     tile_position=(0, c * SLOC))

        # ---- ACT evac of u to bf16 SBUF; even halves first, then the
        # gated sin/cos dma issues, then odd halves (queue-order matters:
        # a gated dma_start blocks later entries of its issuing ring) ----
        u_r = big.tile([NPART, CL], bt, tag="u_r")
        u_i = big.tile([NPART, CL], bt, tag="u_i")
        nc.scalar.copy(u_r[:, 0:HCL], pbu_re[:])
        nc.scalar.copy(u_i[:, 0:HCL], pbu_ie[:])
        nc.scalar.dma_start(out=T1blk[:, HCL:CL], in_=T1_d[:, HCL:CL])
        nc.scalar.dma_start(out=sinblk[:], in_=sin_d)
        nc.scalar.dma_start(out=cosblk[:], in_=cos_d)
        nc.scalar.copy(u_r[:, HCL:CL], pbu_ro[:])
        nc.scalar.copy(u_i[:, HCL:CL], pbu_io[:])

        # ---- modulations (tt 2x) + pair-sums + carries.  The even-half
        # mods run during the odd input DMAs; odd halves + pair sums chase
        # the odd evacs ----
        A = cpool.tile([NPART, 4], dt)
        offs = cpool.tile([NPART, 4], dt)
        Pdum = big.tile([NPART, HCL], bt, tag="Pdum")
        Y = {}
        Pq = {}
        quants = [("1r", T1blk, u_r, 0), ("2r", T2blk, u_r, 1),
                  ("1i", T1blk, u_i, 2), ("2i", T2blk, u_i, 3)]
        for q, T, u, ai in quants:
            Yt = big.tile([NPART, CL], bt, tag=f"Y{q}")
            nc.vector.tensor_mul(Yt[:, 0:HCL], T[:, 0:HCL], u[:, 0:HCL])
            Y[q] = Yt

        def modpair(qi):
            q, T, u, ai = quants[qi]
            Yt = Y[q]
            Pt = big.tile([NPART, HCL], bt, tag=f"P{q}")
            nc.vector.tensor_mul(Yt[:, HCL:CL], T[:, HCL:CL], u[:, HCL:CL])
            nc.vector.tensor_add(Pt[:], Yt[:, 0:HCL], Yt[:, HCL:CL])
            # chunk sums on ACT, off the DVE chain
            nc.scalar.activation(Pdum[:], Pt[:], AF.Identity,
                                 accum_out=A[:, ai:ai + 1])
            Pq[q] = Pt

        S = {}

        def scan(q, ai):
            St = big.tile([NPART, CL], bt, tag=f"S{q}")
            ini = (pbu_re[:, ai:ai + 1] if ai < 2
                   else offs[:, ai:ai + 1])
            bass.BassGpSimd.tensor_tensor_scan(
                nc.vector, St[:, HCL:CL], ones[:], Pq[q][:], ini,
                Alu.mult, Alu.add,
            )
            S[q] = St

        def fix(q):
            # S_even = S_odd - y_odd  (aligned, 2x)
            nc.vector.tensor_sub(S[q][:, 0:HCL], S[q][:, HCL:CL],
                                 Y[q][:, HCL:CL])

        # carry matmuls split per accumulator column so scan k only waits
        # on its own chunk-sum chain, not all four
        modpair(0)                      # Y1r odd, P1r
        nc.tensor.matmul(pbu_re[:, 0:1], Wm[:], A[:, 0:1],
                         start=True, stop=True)
        modpair(1)                      # Y2r odd, P2r
        nc.tensor.matmul(pbu_re[:, 1:2], Wm[:], A[:, 1:2],
                         start=True, stop=True)
        modpair(2)                      # Y1i, P1i  (fills DVE while carry lands)
        scan("1r", 0)
        modpair(3)                      # Y2i, P2i
        nc.tensor.matmul(pbu_ie[:, 0:2], Wm[:], A[:, 2:4],
                         start=True, stop=True)
        nc.scalar.copy(offs[:, 2:4], pbu_ie[:, 0:2])
        scan("2r", 1)
        fix("1r")
        fix("2r")
        m1r = big.tile([NPART, CL], bt, tag="m1r")
        m2r = big.tile([NPART, CL], bt, tag="m2r")
        x_r = big.tile([NPART, CL], bt, tag="x_r")
        nc.vector.tensor_mul(m1r[:], S["1r"][:], sinblk[:])
        nc.vector.tensor_mul(m2r[:], S["2r"][:], cosblk[:])
        nc.vector.tensor_add(x_r[:, 0:HCL], m1r[:, 0:HCL], m2r[:, 0:HCL])
        nc.vector.tensor_add(x_r[:, HCL:CL], m1r[:, HCL:CL], m2r[:, HCL:CL])
        scan("1i", 2)
        scan("2i", 3)
        fix("1i")
        fix("2i")
        m1i = big.tile([NPART, CL], bt, tag="m1i")
        m2i = big.tile([NPART, CL], bt, tag="m2i")
        x_i = big.tile([NPART, CL], bt, tag="x_i")
        nc.vector.tensor_mul(m1i[:], S["1i"][:], sinblk[:])
        nc.vector.tensor_mul(m2i[:], S["2i"][:], cosblk[:])
        nc.vector.tensor_add(x_i[:, 0:HCL], m1i[:, 0:HCL], m2i[:, 0:HCL])
        nc.vector.tensor_add(x_i[:, HCL:CL], m1i[:, HCL:CL], m2i[:, HCL:CL])

        pbu_io_pool.release()
        pbu_ie_pool.release()
        pbu_ro_pool.release()
        pbu_re_pool.release()
        po = tc.alloc_tile_pool(name="po", bufs=4, space="PSUM")

        # ---- projection slabs: out0 = Ctr@x_r (under the i chain),
        #      out1 = Cti@x_i (tail); host sums the slabs.
        # per chunk c the 2048 cols stay [evens 1024 | odds 1024] ----
        for slab, (Wt, x, outd) in enumerate(((Ctr, x_r, out0),
                                              (Cti, x_i, out1))):
            for c in range(FOLD):
                ps = slice(c * SLOC, (c + 1) * SLOC)
                st = stage.tile([128, CL], bt, tag="st")
                for hh in range(2):
                    pt = po.tile([128, 2 * JT], dt, tag="po")
                    for jh in range(2):
                        js = slice(hh * HCL + jh * JT,
                                   hh * HCL + (jh + 1) * JT)
                        nc.tensor.matmul(
                            pt[:, jh * JT:(jh + 1) * JT], Wt[ps, :],
                            x[ps, js], start=True, stop=True,
                            tile_position=(c * SLOC, 0),
                        )
                    # slab0 evacs ride the ACT engine under the i-chain;
                    # slab1 evacs land in the tail where DVE is free
                    if slab == 1 and hh == 0:
                        nc.vector.tensor_copy(st[:, hh * HCL:(hh + 1) * HCL],
                                              pt[:])
                    else:
                        nc.scalar.copy(st[:, hh * HCL:(hh + 1) * HCL], pt[:])
                nc.sync.dma_start(
                    out=outd[:, c * CL:(c + 1) * CL], in_=st[:])
        for p in (po, stage, big, cpool):
            p.release()
    if split_waits:
        _split_matmul_waits(nc, mybir)
    return nc


def _split_matmul_waits(nc, mybir):
    """Hardware instruction structs fit a limited number of embedded sync
    waits; move extra waits onto an inserted same-queue no-op."""
    caps = {"InstMatmult": 1}
    skip = {"InstNoOp", "InstAllEngineBarrier", "InstSync"}
    k = 0
    for bb in nc.main_func.blocks:
        insts = bb.instructions
        i = 0
        while i < len(insts):
            ins = insts[i]
            tn = type(ins).__name__
            if tn not in skip and ins.sync_info is not None:
                cap = caps.get(tn, 1)
                w = list(ins.sync_info.on_wait or [])
                if len(w) > cap:
                    for wj in w[:-cap]:
                        nop = mybir.InstNoOp(
                            name=f"I-mmdep-{k}",
                            engine=ins.engine,
                            ins=[],
                            outs=[],
                            sync_info=mybir.SyncInfo(
                                on_wait=[wj], on_update=[]
                            ),
                        )
                        k += 1
                        insts.insert(i, nop)
                        i += 1
                    ins.sync_info = mybir.SyncInfo(
                        on_wait=w[-cap:], on_update=ins.sync_info.on_update
                    )
            i += 1


def _eo_permute(a):
    """per 2048-col chunk: natural t' order -> [evens 1024 | odds 1024]."""
    r, n = a.shape
    nch = n // CL
    return np.ascontiguousarray(
        a.reshape(r, nch, CL // 2, 2).transpose(0, 1, 3, 2).reshape(r, n))


def _eo_unpermute(a):
    r, n = a.shape
    nch = n // CL
    return np.ascontiguousarray(
        a.reshape(r, nch, 2, CL // 2).transpose(0, 1, 3, 2).reshape(r, n))


def _host_prep(inputs):
    import ml_dtypes
    bf16 = ml_dtypes.bfloat16
    f32 = np.float32

    inp32 = np.asarray(inputs["input_sequence"], np.float32)
    inpT = _eo_permute(np.ascontiguousarray(inp32.T)).astype(bf16)
    A = np.maximum(np.asarray(inputs["A_diag_raw"], np.float64), 0.0)
    s = 1.0 / (1.0 + np.exp(-np.asarray(inputs["steps_raw"], np.float64)))
    Br = np.asarray(inputs["B_real"], np.float64)
    Bi = np.asarray(inputs["B_img"], np.float64)
    Cr = np.asarray(inputs["C_real"], np.float64)
    Ci = np.asarray(inputs["C_img"], np.float64)

    costh = 1.0 - s * s * A / 2.0
    sinth = np.sqrt(np.maximum(1.0 - costh * costh, 1e-300))
    theta = np.arctan2(sinth, costh)
    gamma = (s - s * s * A / 2.0) / sinth

    q = np.arange(NPART)
    Wm = ((q[:, None] % SLOC == q[None, :] % SLOC)
          & (q[:, None] // SLOC < q[None, :] // SLOC)).astype(f32)

    tvec = np.arange(CL, dtype=np.float64)
    twopi = 2.0 * np.pi

    in_maps = []
    for k in range(NCORES):
        sl = slice(k * SLOC, (k + 1) * SLOC)
        th = theta[sl]
        gm = gamma[sl]
        BCC = np.empty((128, 320), bf16)
        BCC[:, 0:SLOC] = (s[sl, None] * Br[sl]).T.astype(bf16)
        BCC[:, SLOC:2 * SLOC] = (s[sl, None] * Bi[sl]).T.astype(bf16)
        BCC[:, 2 * SLOC:2 * SLOC + H] = np.tile(
            Cr[:, sl].T, (FOLD, 1)).astype(bf16)
        BCC[:, 2 * SLOC + H:] = np.tile(
            -Ci[:, sl].T, (FOLD, 1)).astype(bf16)

        # tables per partition q = c*SLOC + s at global time t = c*CL + j
        ang = np.empty((NPART, CL), np.float64)
        for c in range(FOLD):
            ang[c * SLOC:(c + 1) * SLOC] = np.mod(
                (c * CL + tvec)[None, :] * th[:, None], twopi)
        sinA = np.sin(ang)
        cosA = np.cos(ang)
        gq = np.tile(gm, FOLD)[:, None]
        T1 = gq * cosA + sinA
        T2 = cosA - gq * sinA

        m = {"inpT": inpT, "BCC": BCC, "Wm": Wm}
        for nm, tb in (("T1blk", T1), ("T2blk", T2),
                       ("sinblk", sinA), ("cosblk", cosA)):
            m[nm] = _eo_permute(np.ascontiguousarray(tb)).astype(bf16)
        in_maps.append(m)
    return in_maps


LAST_RESULTS = None


def kernel(**inputs) -> np.ndarray:
    global LAST_RESULTS
    from concourse.bass_utils import run_bass_kernel_spmd

    if "nc" not in _CACHE:
        _CACHE["nc"] = _build_bass()
    nc = _CACHE["nc"]

    in_maps = _host_prep(inputs)
    res = run_bass_kernel_spmd(nc, in_maps, core_ids=list(range(NCORES)))
    LAST_RESULTS = res
    part = np.zeros((H, L), np.float32)
    for r in res.results:
        part += np.asarray(r["out0"], np.float32)
        part += np.asarray(r["out1"], np.float32)
    out = np.ascontiguousarray(_eo_unpermute(part).T)
    out += (np.asarray(inputs["input_sequence"], np.float32)
            * np.asarray(inputs["D"], np.float32)[None, :])
    return out


# revision 22
# speedup vs baseline: 1.0404x; 1.0404x over previous
"""LinOSS layer Trainium2 kernel, v4.2.

Math (same closed form as v3): the per-state 2x2 recurrence has eigenvalues
e^{+-i theta}; the scanned state collapses to rank-2 modulated prefix sums

    u     = s * Bu                     (s folded into B on host)
    E     = cumsum(T1 * u);  F = cumsum(T2 * u)     per complex part
    x_t   = sin(t th) * E_t + cos(t th) * F_t
    T1    = gamma*cos + sin;  T2 = cos - gamma*sin

Structure (keeps the 128 = 4 time-chunks x 32 states partition fold):
  - EVEN/ODD TIME SPLIT, done on the host: inpT columns are permuted per
    2048-chunk to [evens 1024 | odds 1024]; all tables pre-blocked to match.
    The DVE scan (2 cycles/col, no perf modes) then only runs over the
    1024 pair-sums P_j = y_{2j} + y_{2j+1}:
        S_{2j+1} = seed + cumsum(P)_j          (scan, half length)
        S_{2j}   = S_{2j+1} - y_{2j+1}         (aligned 2x tensor_tensor sub)
    halving the dominant scan cost.
  - u is evacuated from PSUM to bf16 SBUF by the ACT engine (4 separate
    e/o PSUM tiles so the even evacs start mid-load), so modulations and
    demodulations are all-bf16 full-width [128,2048] tensor_tensor ops in
    DVE 2x mode.
  - per-chunk carry sums come from ACT activation(Identity, accum_out=..)
    re-reading the pair-sums (off the DVE critical path); Wm matmul turns
    them into scan initial values (v3 mechanism).
  - DMA issue cost is ~0.6us PER dma_start on a HWDGE ring (measured): the
    issue stream is split across BOTH rings (sync + scalar), small tensors
    are packed into one transfer, and out-DMAs batch 2 evacs each.
  - projection/output as v3: two slabs out0/out1, host sums + un-permutes.
"""

import numpy as np

L, H, P = 8192, 128, 256
NCORES = 8
SLOC = P // NCORES          # states per core
FOLD = 4                    # time chunks folded into partitions
CL = L // FOLD              # 2048 free columns per partition row
HCL = CL // 2               # 1024 columns per even/odd half
NPART = FOLD * SLOC         # 128
JT = 512                    # matmul j-tile width

_CACHE: dict = {}


def _build_bass(split_waits=True):
    import concourse.bass as bass
    import concourse.mybir as mybir
    import concourse.tile as tile

    dt = mybir.dt.float32
    bt = mybir.dt.bfloat16
    Alu = mybir.AluOpType
    AF = mybir.ActivationFunctionType

    nc = bass.Bass(
        trn_type="TRN2",
        target_bir_lowering=False,
        debug=False,
        num_devices=NCORES,
    )

    inpT_d = nc.dram_tensor("inpT", [H, L], bt, kind="ExternalInput").ap()
    # packed: Bt [*,0:64] | Ctr [*,64:192] | Cti [*,192:320]
    BCC_d = nc.dram_tensor("BCC", [128, 320], bt, kind="ExternalInput").ap()
    Wm_d = nc.dram_tensor("Wm", [NPART, NPART], dt, kind="ExternalInput").ap()
    T1_d = nc.dram_tensor("T1blk", [NPART, CL], bt, kind="ExternalInput").ap()
    T2_d = nc.dram_tensor("T2blk", [NPART, CL], bt, kind="ExternalInput").ap()
    sin_d = nc.dram_tensor("sinblk", [NPART, CL], bt, kind="ExternalInput").ap()
    cos_d = nc.dram_tensor("cosblk", [NPART, CL], bt, kind="ExternalInput").ap()
    out0 = nc.dram_tensor("out0", [H, L], bt, kind="ExternalOutput").ap()
    out1 = nc.dram_tensor("out1", [H, L], bt, kind="ExternalOutput").ap()

    with tile.TileContext(nc) as tc:
        cpool = tc.alloc_tile_pool(name="const", bufs=1)
        big = tc.alloc_tile_pool(name="big", bufs=1)
        stage = tc.alloc_tile_pool(name="stage", bufs=4)
        pbu_re_pool = tc.alloc_tile_pool(name="pbu_re", bufs=1, space="PSUM")
        pbu_ro_pool = tc.alloc_tile_pool(name="pbu_ro", bufs=1, space="PSUM")
        pbu_ie_pool = tc.alloc_tile_pool(name="pbu_ie", bufs=1, space="PSUM")
        pbu_io_pool = tc.alloc_tile_pool(name="pbu_io", bufs=1, space="PSUM")

        # ---- loads; issue cost is ~0.6us per dma_start per ring, so the
        # stream is split across the two HWDGE rings (sync + scalar) ----
        # sync ring: the 8 inpT chunks (evens first), then T2/cos tables.
        inpT = big.tile([128, L], bt, tag="inpT")
        KB = L // 8  # 1024-col dma blocks; evens are dram blocks 0,2,4,6
        for k8 in (0, 2, 4, 6, 1, 3, 5, 7):
            nc.sync.dma_start(
                out=inpT[:, k8 * KB:(k8 + 1) * KB],
                in_=inpT_d[:, k8 * KB:(k8 + 1) * KB],
            )
        # tables are WAR-gated on inpT progress (gpsimd dummy reading an
        # inpT block + writing a dummy slot of the table tile) so the
        # fair-shared DMA bandwidth stays on the critical input chunks
        def gate(name, gate_k8, col=0):
            # the dummy write must overlap the region the gated DMA writes
            dummy = big.tile([NPART, CL], bt, tag=name)
            gd = cpool.tile([1, 8], dt, tag=f"g_{name}_{col}")
            nc.gpsimd.memset(dummy[0:1, col:col + 8], 0.0)
            nc.gpsimd.tensor_tensor(
                gd[:], dummy[0:1, col:col + 8],
                inpT[0:1, gate_k8 * KB:gate_k8 * KB + 8], Alu.add)
            real = big.tile([NPART, CL], bt, tag=name)
            return real

        T2blk = gate("T2blk", 6)       # after 4th even block
        nc.sync.dma_start(out=T2blk[:], in_=T2_d)
        # scalar ring: small packed tensors + T1/sin/cos tables.
        BCC = cpool.tile([128, 320], bt)
        nc.scalar.dma_start(out=BCC[:], in_=BCC_d)
        Wm = cpool.tile([NPART, NPART], dt)
        nc.scalar.dma_start(out=Wm[:], in_=Wm_d)
        T1blk = gate("T1blk", 4)       # after 3rd even block
        nc.scalar.dma_start(out=T1blk[:], in_=T1_d)
        sinblk = gate("sinblk", 3)     # after 2nd odd block
        cosblk = gate("cosblk", 3)
        Ctr = BCC[:, 2 * SLOC:2 * SLOC + H]
        Cti = BCC[:, 2 * SLOC + H:2 * SLOC + 2 * H]

        ones = cpool.tile([NPART, HCL], bt)
        nc.vector.memset(ones[:], 1.0)

        # ---- Bu matmuls; four separate psum tiles so the even-half evacs
        # can start while the odd chunks are still streaming ----
        pbu_re = pbu_re_pool.tile([NPART, HCL], dt, tag="bu_re")
        pbu_ro = pbu_ro_pool.tile([NPART, HCL], dt, tag="bu_ro")
        pbu_ie = pbu_ie_pool.tile([NPART, HCL], dt, tag="bu_ie")
        pbu_io = pbu_io_pool.tile([NPART, HCL], dt, tag="bu_io")
        bs_r = slice(0, SLOC)
        bs_i = slice(SLOC, 2 * SLOC)
        for half, (pr, pi) in ((0, (pbu_re, pbu_ie)), (1, (pbu_ro, pbu_io))):
            for c in range(FOLD):
                ps = slice(c * SLOC, (c + 1) * SLOC)
                for jh in range(2):
                    rhs = inpT[:, c * CL + half * HCL + jh * JT:
                               c * CL + half * HCL + (jh + 1) * JT]
                    js = slice(jh * JT, (jh + 1) * JT)
                    nc.tensor.matmul(pr[ps, js], BCC[:, bs_r], rhs,
                                     start=True, stop=True,
                                     tile_position=(0, c * SLOC))
                    nc.tensor.matmul(pi[ps, js], BCC[:, bs_i], rhs,
                                     start=True, stop=True,
                                     tile_position=(0, c * SLOC))

        # ---- ACT evac of u to bf16 SBUF; even halves first, then the
        # gated sin/cos dma issues, then odd halves (queue-order matters:
        # a gated dma_start blocks later entries of its issuing ring) ----
        u_r = big.tile([NPART, CL], bt, tag="u_r")
        u_i = big.tile([NPART, CL], bt, tag="u_i")
        nc.scalar.copy(u_r[:, 0:HCL], pbu_re[:])
        nc.scalar.copy(u_i[:, 0:HCL], pbu_ie[:])
        nc.scalar.dma_start(out=sinblk[:], in_=sin_d)
        nc.scalar.dma_start(out=cosblk[:], in_=cos_d)
        nc.scalar.copy(u_r[:, HCL:CL], pbu_ro[:])
        nc.scalar.copy(u_i[:, HCL:CL], pbu_io[:])

        # ---- modulations (tt 2x) + pair-sums + carries.  The even-half
        # mods run during the odd input DMAs; odd halves + pair sums chase
        # the odd evacs ----
        A = cpool.tile([NPART, 4], dt)
        offs = cpool.tile([NPART, 4], dt)
        Pdum = big.tile([NPART, HCL], bt, tag="Pdum")
        Y = {}
        Pq = {}
        quants = [("1r", T1blk, u_r, 0), ("2r", T2blk, u_r, 1),
                  ("1i", T1blk, u_i, 2), ("2i", T2blk, u_i, 3)]

        def modpair(qi):
            # per-quantity Ye, Yo, P back-to-back: P completes as soon as
            # this quantity's inputs land, so its carry chain starts early
            q, T, u, ai = quants[qi]
            Yt = big.tile([NPART, CL], bt, tag=f"Y{q}")
            Pt = big.tile([NPART, HCL], bt, tag=f"P{q}")
            nc.vector.tensor_mul(Yt[:, 0:HCL], T[:, 0:HCL], u[:, 0:HCL])
            nc.vector.tensor_mul(Yt[:, HCL:CL], T[:, HCL:CL], u[:, HCL:CL])
            nc.vector.tensor_add(Pt[:], Yt[:, 0:HCL], Yt[:, HCL:CL])
            # chunk sums on ACT, off the DVE chain
            nc.scalar.activation(Pdum[:], Pt[:], AF.Identity,
                                 accum_out=A[:, ai:ai + 1])
            Y[q] = Yt
            Pq[q] = Pt

        S = {}

        def scan(q, ai):
            St = big.tile([NPART, CL], bt, tag=f"S{q}")
            ini = (pbu_re[:, ai:ai + 1] if ai < 2
                   else offs[:, ai:ai + 1])
            bass.BassGpSimd.tensor_tensor_scan(
                nc.vector, St[:, HCL:CL], ones[:], Pq[q][:], ini,
                Alu.mult, Alu.add,
            )
            S[q] = St

        def fix(q):
            # S_even = S_odd - y_odd  (aligned, 2x)
            nc.vector.tensor_sub(S[q][:, 0:HCL], S[q][:, HCL:CL],
                                 Y[q][:, HCL:CL])

        # carry matmuls + seed copies split per accumulator column so each
        # scan only waits on its own chunk-sum chain, not all four
        modpair(0)                      # Y1r, P1r
        nc.tensor.matmul(pbu_re[:, 0:1], Wm[:], A[:, 0:1],
                         start=True, stop=True)
        modpair(1)                      # Y2r, P2r
        nc.tensor.matmul(pbu_re[:, 1:2], Wm[:], A[:, 1:2],
                         start=True, stop=True)
        scan("1r", 0)
        scan("2r", 1)
        # i-pair mods + carries; the r fix/demod block below fills the DVE
        # while the i carry chains land on ACT/PE
        modpair(2)                      # Y1i, P1i
        nc.tensor.matmul(pbu_ie[:, 0:1], Wm[:], A[:, 2:3],
                         start=True, stop=True)
        nc.scalar.copy(offs[:, 2:3], pbu_ie[:, 0:1])
        modpair(3)                      # Y2i, P2i
        nc.tensor.matmul(pbu_ie[:, 1:2], Wm[:], A[:, 3:4],
                         start=True, stop=True)
        nc.scalar.copy(offs[:, 3:4], pbu_ie[:, 1:2])
        fix("1r")
        fix("2r")
        m1r = big.tile([NPART, CL], bt, tag="m1r")
        m2r = big.tile([NPART, CL], bt, tag="m2r")
        x_r = big.tile([NPART, CL], bt, tag="x_r")
        nc.vector.tensor_mul(m1r[:], S["1r"][:], sinblk[:])
        nc.vector.tensor_mul(m2r[:], S["2r"][:], cosblk[:])
        nc.vector.tensor_add(x_r[:, 0:HCL], m1r[:, 0:HCL], m2r[:, 0:HCL])
        nc.vector.tensor_add(x_r[:, HCL:CL], m1r[:, HCL:CL], m2r[:, HCL:CL])
        scan("1i", 2)
        scan("2i", 3)
        fix("1i")
        fix("2i")
        m1i = big.tile([NPART, CL], bt, tag="m1i")
        m2i = big.tile([NPART, CL], bt, tag="m2i")
        x_i = big.tile([NPART, CL], bt, tag="x_i")
        nc.vector.tensor_mul(m1i[:], S["1i"][:], sinblk[:])
        nc.vector.tensor_mul(m2i[:], S["2i"][:], cosblk[:])
        nc.vector.tensor_add(x_i[:, 0:HCL], m1i[:, 0:HCL], m2i[:, 0:HCL])
        nc.vector.tensor_add(x_i[:, HCL:CL], m1i[:, HCL:CL], m2i[:, HCL:CL])

        pbu_io_pool.release()
        pbu_ie_pool.release()
        pbu_ro_pool.release()
        pbu_re_pool.release()
        po = tc.alloc_tile_pool(name="po", bufs=4, space="PSUM")

        # ---- projection slabs: out0 = Ctr@x_r (under the i chain),
        #      out1 = Cti@x_i (tail); host sums the slabs.
        # per chunk c the 2048 cols stay [evens 1024 | odds 1024] ----
        for slab, (Wt, x, outd) in enumerate(((Ctr, x_r, out0),
                                              (Cti, x_i, out1))):
            for c in range(FOLD):
                ps = slice(c * SLOC, (c + 1) * SLOC)
                st = stage.tile([128, CL], bt, tag="st")
                for hh in range(2):
                    pt = po.tile([128, 2 * JT], dt, tag="po")
                    for jh in range(2):
                        js = slice(hh * HCL + jh * JT,
                                   hh * HCL + (jh + 1) * JT)
                        nc.tensor.matmul(
                            pt[:, jh * JT:(jh + 1) * JT], Wt[ps, :],
                            x[ps, js], start=True, stop=True,
                            tile_position=(c * SLOC, 0),
                        )
                    # slab0 evacs ride the ACT engine under the i-chain;
                    # slab1 evacs land in the tail where DVE is free
                    if slab == 1 and hh == 0:
                        nc.vector.tensor_copy(st[:, hh * HCL:(hh + 1) * HCL],
                                              pt[:])
                    else:
                        nc.scalar.copy(st[:, hh * HCL:(hh + 1) * HCL], pt[:])
                nc.sync.dma_start(
                    out=outd[:, c * CL:(c + 1) * CL], in_=st[:])
        for p in (po, stage, big, cpool):
            p.release()
    if split_waits:
        _split_matmul_waits(nc, mybir)
    return nc


def _split_matmul_waits(nc, mybir):
    """Hardware instruction structs fit a limited number of embedded sync
    waits; move extra waits onto an inserted same-queue no-op."""
    caps = {"InstMatmult": 1}
    skip = {"InstNoOp", "InstAllEngineBarrier", "InstSync"}
    k = 0
    for bb in nc.main_func.blocks:
        insts = bb.instructions
        i = 0
        while i < len(insts):
            ins = insts[i]
            tn = type(ins).__name__
            if tn not in skip and ins.sync_info is not None:
                cap = caps.get(tn, 1)
                w = list(ins.sync_info.on_wait or [])
                if len(w) > cap:
                    for wj in w[:-cap]:
                        nop = mybir.InstNoOp(
                            name=f"I-mmdep-{k}",
                            engine=ins.engine,
                            ins=[],
                            outs=[],
                            sync_info=mybir.SyncInfo(
                                on_wait=[wj], on_update=[]
                            ),
                        )
                        k += 1
                        insts.insert(i, nop)
                        i += 1
                    ins.sync_info = mybir.SyncInfo(
                        on_wait=w[-cap:], on_update=ins.sync_info.on_update
                    )
            i += 1


def _eo_permute(a):
    """per 2048-col chunk: natural t' order -> [evens 1024 | odds 1024]."""
    r, n = a.shape
    nch = n // CL
    return np.ascontiguousarray(
        a.reshape(r, nch, CL // 2, 2).transpose(0, 1, 3, 2).reshape(r, n))


def _eo_unpermute(a):
    r, n = a.shape
    nch = n // CL
    return np.ascontiguousarray(
        a.reshape(r, nch, 2, CL // 2).transpose(0, 1, 3, 2).reshape(r, n))


def _host_prep(inputs):
    import ml_dtypes
    bf16 = ml_dtypes.bfloat16
    f32 = np.float32

    inp32 = np.asarray(inputs["input_sequence"], np.float32)
    inpT = _eo_permute(np.ascontiguousarray(inp32.T)).astype(bf16)
    A = np.maximum(np.asarray(inputs["A_diag_raw"], np.float64), 0.0)
    s = 1.0 / (1.0 + np.exp(-np.asarray(inputs["steps_raw"], np.float64)))
    Br = np.asarray(inputs["B_real"], np.float64)
    Bi = np.asarray(inputs["B_img"], np.float64)
    Cr = np.asarray(inputs["C_real"], np.float64)
    Ci = np.asarray(inputs["C_img"], np.float64)

    costh = 1.0 - s * s * A / 2.0
    sinth = np.sqrt(np.maximum(1.0 - costh * costh, 1e-300))
    theta = np.arctan2(sinth, costh)
    gamma = (s - s * s * A / 2.0) / sinth

    q = np.arange(NPART)
    Wm = ((q[:, None] % SLOC == q[None, :] % SLOC)
          & (q[:, None] // SLOC < q[None, :] // SLOC)).astype(f32)

    tvec = np.arange(CL, dtype=np.float64)
    twopi = 2.0 * np.pi

    in_maps = []
    for k in range(NCORES):
        sl = slice(k * SLOC, (k + 1) * SLOC)
        th = theta[sl]
        gm = gamma[sl]
        BCC = np.empty((128, 320), bf16)
        BCC[:, 0:SLOC] = (s[sl, None] * Br[sl]).T.astype(bf16)
        BCC[:, SLOC:2 * SLOC] = (s[sl, None] * Bi[sl]).T.astype(bf16)
        BCC[:, 2 * SLOC:2 * SLOC + H] = np.tile(
            Cr[:, sl].T, (FOLD, 1)).astype(bf16)
        BCC[:, 2 * SLOC + H:] = np.tile(
            -Ci[:, sl].T, (FOLD, 1)).astype(bf16)

        # tables per partition q = c*SLOC + s at global time t = c*CL + j
        ang = np.empty((NPART, CL), np.float64)
        for c in range(FOLD):
            ang[c * SLOC:(c + 1) * SLOC] = np.mod(
                (c * CL + tvec)[None, :] * th[:, None], twopi)
        sinA = np.sin(ang)
        cosA = np.cos(ang)
        gq = np.tile(gm, FOLD)[:, None]
        T1 = gq * cosA + sinA
        T2 = cosA - gq * sinA

        m = {"inpT": inpT, "BCC": BCC, "Wm": Wm}
        for nm, tb in (("T1blk", T1), ("T2blk", T2),
                       ("sinblk", sinA), ("cosblk", cosA)):
            m[nm] = _eo_permute(np.ascontiguousarray(tb)).astype(bf16)
        in_maps.append(m)
    return in_maps


LAST_RESULTS = None


def kernel(**inputs) -> np.ndarray:
    global LAST_RESULTS
    from concourse.bass_utils import run_bass_kernel_spmd

    if "nc" not in _CACHE:
        _CACHE["nc"] = _build_bass()
    nc = _CACHE["nc"]

    in_maps = _host_prep(inputs)
    res = run_bass_kernel_spmd(nc, in_maps, core_ids=list(range(NCORES)))
    LAST_RESULTS = res
    part = np.zeros((H, L), np.float32)
    for r in res.results:
        part += np.asarray(r["out0"], np.float32)
        part += np.asarray(r["out1"], np.float32)
    out = np.ascontiguousarray(_eo_unpermute(part).T)
    out += (np.asarray(inputs["input_sequence"], np.float32)
            * np.asarray(inputs["D"], np.float32)[None, :])
    return out


# revision 26
# speedup vs baseline: 1.0779x; 1.0361x over previous
"""LinOSS layer Trainium2 kernel, v4.2.

Math (same closed form as v3): the per-state 2x2 recurrence has eigenvalues
e^{+-i theta}; the scanned state collapses to rank-2 modulated prefix sums

    u     = s * Bu                     (s folded into B on host)
    E     = cumsum(T1 * u);  F = cumsum(T2 * u)     per complex part
    x_t   = sin(t th) * E_t + cos(t th) * F_t
    T1    = gamma*cos + sin;  T2 = cos - gamma*sin

Structure (keeps the 128 = 4 time-chunks x 32 states partition fold):
  - EVEN/ODD TIME SPLIT, done on the host: inpT columns are permuted per
    2048-chunk to [evens 1024 | odds 1024]; all tables pre-blocked to match.
    The DVE scan (2 cycles/col, no perf modes) then only runs over the
    1024 pair-sums P_j = y_{2j} + y_{2j+1}:
        S_{2j+1} = seed + cumsum(P)_j          (scan, half length)
        S_{2j}   = S_{2j+1} - y_{2j+1}         (aligned 2x tensor_tensor sub)
    halving the dominant scan cost.
  - u is evacuated from PSUM to bf16 SBUF by the ACT engine (4 separate
    e/o PSUM tiles so the even evacs start mid-load), so modulations and
    demodulations are all-bf16 full-width [128,2048] tensor_tensor ops in
    DVE 2x mode.
  - per-chunk carry sums come from ACT activation(Identity, accum_out=..)
    re-reading the pair-sums (off the DVE critical path); Wm matmul turns
    them into scan initial values (v3 mechanism).
  - DMA issue cost is ~0.6us PER dma_start on a HWDGE ring (measured): the
    issue stream is split across BOTH rings (sync + scalar), small tensors
    are packed into one transfer, and out-DMAs batch 2 evacs each.
  - projection/output as v3: two slabs out0/out1, host sums + un-permutes.
"""

import numpy as np

L, H, P = 8192, 128, 256
NCORES = 8
SLOC = P // NCORES          # states per core
FOLD = 4                    # time chunks folded into partitions
CL = L // FOLD              # 2048 free columns per partition row
HCL = CL // 2               # 1024 columns per even/odd half
NPART = FOLD * SLOC         # 128
JT = 512                    # matmul j-tile width

_CACHE: dict = {}


def _build_bass(split_waits=True):
    import concourse.bass as bass
    import concourse.mybir as mybir
    import concourse.tile as tile

    dt = mybir.dt.float32
    bt = mybir.dt.bfloat16
    Alu = mybir.AluOpType
    AF = mybir.ActivationFunctionType

    nc = bass.Bass(
        trn_type="TRN2",
        target_bir_lowering=False,
        debug=False,
        num_devices=NCORES,
    )

    inpT_d = nc.dram_tensor("inpT", [H, L], bt, kind="ExternalInput").ap()
    # packed: Bt [*,0:64] | Ctr [*,64:192] | Cti [*,192:320]
    BCC_d = nc.dram_tensor("BCC", [128, 320], bt, kind="ExternalInput").ap()
    Wm_d = nc.dram_tensor("Wm", [NPART, NPART], dt, kind="ExternalInput").ap()
    T1_d = nc.dram_tensor("T1blk", [NPART, CL], bt, kind="ExternalInput").ap()
    T2_d = nc.dram_tensor("T2blk", [NPART, CL], bt, kind="ExternalInput").ap()
    sin_d = nc.dram_tensor("sinblk", [NPART, CL], bt, kind="ExternalInput").ap()
    cos_d = nc.dram_tensor("cosblk", [NPART, CL], bt, kind="ExternalInput").ap()
    out0 = nc.dram_tensor("out0", [H, L], bt, kind="ExternalOutput").ap()
    out1 = nc.dram_tensor("out1", [H, L], bt, kind="ExternalOutput").ap()

    with tile.TileContext(nc) as tc:
        cpool = tc.alloc_tile_pool(name="const", bufs=1)
        big = tc.alloc_tile_pool(name="big", bufs=1)
        stage = tc.alloc_tile_pool(name="stage", bufs=4)
        pbu_re_pool = tc.alloc_tile_pool(name="pbu_re", bufs=1, space="PSUM")
        pbu_ro_pool = tc.alloc_tile_pool(name="pbu_ro", bufs=1, space="PSUM")
        pbu_ie_pool = tc.alloc_tile_pool(name="pbu_ie", bufs=1, space="PSUM")
        pbu_io_pool = tc.alloc_tile_pool(name="pbu_io", bufs=1, space="PSUM")

        # ---- loads; issue cost is ~0.6us per dma_start per ring, so the
        # stream is split across the two HWDGE rings (sync + scalar) ----
        # sync ring: the 8 inpT chunks (evens first), then T2/cos tables.
        inpT = big.tile([128, L], bt, tag="inpT")
        T1blk = big.tile([NPART, CL], bt, tag="T1blk")
        KB = L // 8  # 1024-col dma blocks; evens are dram blocks 0,2,4,6
        for k8 in (0, 2, 4, 6):
            nc.sync.dma_start(
                out=inpT[:, k8 * KB:(k8 + 1) * KB],
                in_=inpT_d[:, k8 * KB:(k8 + 1) * KB],
            )
        # T1 rides the sync ring between the even and odd input chunks:
        # it gates the first modulations, so it must land with the odds
        nc.sync.dma_start(out=T1blk[:], in_=T1_d)
        for k8 in (1, 3, 5, 7):
            nc.sync.dma_start(
                out=inpT[:, k8 * KB:(k8 + 1) * KB],
                in_=inpT_d[:, k8 * KB:(k8 + 1) * KB],
            )
        # tables are WAR-gated on inpT progress (gpsimd dummy reading an
        # inpT block + writing a dummy slot of the table tile) so the
        # fair-shared DMA bandwidth stays on the critical input chunks
        def gate(name, gate_k8, col=0):
            # the dummy write must overlap the region the gated DMA writes
            dummy = big.tile([NPART, CL], bt, tag=name)
            gd = cpool.tile([1, 8], dt, tag=f"g_{name}_{col}")
            nc.gpsimd.memset(dummy[0:1, col:col + 8], 0.0)
            nc.gpsimd.tensor_tensor(
                gd[:], dummy[0:1, col:col + 8],
                inpT[0:1, gate_k8 * KB:gate_k8 * KB + 8], Alu.add)
            real = big.tile([NPART, CL], bt, tag=name)
            return real

        T2blk = gate("T2blk", 6)       # after 4th even block
        nc.sync.dma_start(out=T2blk[:], in_=T2_d)
        # scalar ring: small packed tensors + T1/sin/cos tables.
        BCC = cpool.tile([128, 320], bt)
        nc.scalar.dma_start(out=BCC[:], in_=BCC_d)
        Wm = cpool.tile([NPART, NPART], dt)
        nc.scalar.dma_start(out=Wm[:], in_=Wm_d)
        sinblk = gate("sinblk", 3)     # after 2nd odd block
        cosblk = gate("cosblk", 3)
        Ctr = BCC[:, 2 * SLOC:2 * SLOC + H]
        Cti = BCC[:, 2 * SLOC + H:2 * SLOC + 2 * H]

        ones = cpool.tile([NPART, HCL], bt)
        nc.vector.memset(ones[:], 1.0)

        # ---- Bu matmuls; four separate psum tiles so the even-half evacs
        # can start while the odd chunks are still streaming ----
        pbu_re = pbu_re_pool.tile([NPART, HCL], dt, tag="bu_re")
        pbu_ro = pbu_ro_pool.tile([NPART, HCL], dt, tag="bu_ro")
        pbu_ie = pbu_ie_pool.tile([NPART, HCL], dt, tag="bu_ie")
        pbu_io = pbu_io_pool.tile([NPART, HCL], dt, tag="bu_io")
        bs_r = slice(0, SLOC)
        bs_i = slice(SLOC, 2 * SLOC)
        for half, (pr, pi) in ((0, (pbu_re, pbu_ie)), (1, (pbu_ro, pbu_io))):
            for c in range(FOLD):
                ps = slice(c * SLOC, (c + 1) * SLOC)
                for jh in range(2):
                    rhs = inpT[:, c * CL + half * HCL + jh * JT:
                               c * CL + half * HCL + (jh + 1) * JT]
                    js = slice(jh * JT, (jh + 1) * JT)
                    nc.tensor.matmul(pr[ps, js], BCC[:, bs_r], rhs,
                                     start=True, stop=True,
                                     tile_position=(0, c * SLOC))
                    nc.tensor.matmul(pi[ps, js], BCC[:, bs_i], rhs,
                                     start=True, stop=True,
                                     tile_position=(0, c * SLOC))

        # ---- ACT evac of u to bf16 SBUF; even halves first, then the
        # gated sin/cos dma issues, then odd halves (queue-order matters:
        # a gated dma_start blocks later entries of its issuing ring) ----
        u_r = big.tile([NPART, CL], bt, tag="u_r")
        u_i = big.tile([NPART, CL], bt, tag="u_i")
        nc.scalar.copy(u_r[:, 0:HCL], pbu_re[:])
        nc.scalar.copy(u_i[:, 0:HCL], pbu_ie[:])
        nc.scalar.dma_start(out=sinblk[:], in_=sin_d)
        nc.scalar.dma_start(out=cosblk[:], in_=cos_d)
        nc.scalar.copy(u_r[:, HCL:CL], pbu_ro[:])
        nc.scalar.copy(u_i[:, HCL:CL], pbu_io[:])

        # ---- modulations (tt 2x) + pair-sums + carries.  The even-half
        # mods run during the odd input DMAs; odd halves + pair sums chase
        # the odd evacs ----
        A = cpool.tile([NPART, 4], dt)
        offs = cpool.tile([NPART, 4], dt)
        Pdum = big.tile([NPART, HCL], bt, tag="Pdum")
        Y = {}
        Pq = {}
        quants = [("1r", T1blk, u_r, 0), ("2r", T2blk, u_r, 1),
                  ("1i", T1blk, u_i, 2), ("2i", T2blk, u_i, 3)]
        for q, T, u, ai in quants:
            Yt = big.tile([NPART, CL], bt, tag=f"Y{q}")
            nc.vector.tensor_mul(Yt[:, 0:HCL], T[:, 0:HCL], u[:, 0:HCL])
            Y[q] = Yt

        def modpair(qi):
            q, T, u, ai = quants[qi]
            Yt = Y[q]
            Pt = big.tile([NPART, HCL], bt, tag=f"P{q}")
            nc.vector.tensor_mul(Yt[:, HCL:CL], T[:, HCL:CL], u[:, HCL:CL])
            nc.vector.tensor_add(Pt[:], Yt[:, 0:HCL], Yt[:, HCL:CL])
            # chunk sums on ACT, off the DVE chain
            nc.scalar.activation(Pdum[:], Pt[:], AF.Identity,
                                 accum_out=A[:, ai:ai + 1])
            Pq[q] = Pt

        S = {}

        def scan(q, ai):
            St = big.tile([NPART, CL], bt, tag=f"S{q}")
            ini = (pbu_re[:, ai:ai + 1] if ai < 2
                   else offs[:, ai:ai + 1])
            bass.BassGpSimd.tensor_tensor_scan(
                nc.vector, St[:, HCL:CL], ones[:], Pq[q][:], ini,
                Alu.mult, Alu.add,
            )
            S[q] = St

        def fix(q):
            # S_even = S_odd - y_odd  (aligned, 2x)
            nc.vector.tensor_sub(S[q][:, 0:HCL], S[q][:, HCL:CL],
                                 Y[q][:, HCL:CL])

        modpair(0)                      # Y1r odd, P1r
        modpair(1)                      # Y2r odd, P2r
        nc.tensor.matmul(pbu_re[:, 0:2], Wm[:], A[:, 0:2],
                         start=True, stop=True)
        modpair(2)                      # Y1i, P1i  (fills DVE while carry lands)
        scan("1r", 0)
        modpair(3)                      # Y2i, P2i
        nc.tensor.matmul(pbu_ie[:, 0:2], Wm[:], A[:, 2:4],
                         start=True, stop=True)
        nc.scalar.copy(offs[:, 2:4], pbu_ie[:, 0:2])
        scan("2r", 1)
        fix("1r")
        fix("2r")
        m1r = big.tile([NPART, CL], bt, tag="m1r")
        m2r = big.tile([NPART, CL], bt, tag="m2r")
        x_r = big.tile([NPART, CL], bt, tag="x_r")
        nc.vector.tensor_mul(m1r[:], S["1r"][:], sinblk[:])
        nc.vector.tensor_mul(m2r[:], S["2r"][:], cosblk[:])
        nc.vector.tensor_add(x_r[:, 0:HCL], m1r[:, 0:HCL], m2r[:, 0:HCL])
        nc.vector.tensor_add(x_r[:, HCL:CL], m1r[:, HCL:CL], m2r[:, HCL:CL])
        scan("1i", 2)
        scan("2i", 3)
        fix("1i")
        fix("2i")
        m1i = big.tile([NPART, CL], bt, tag="m1i")
        m2i = big.tile([NPART, CL], bt, tag="m2i")
        x_i = big.tile([NPART, CL], bt, tag="x_i")
        nc.vector.tensor_mul(m1i[:], S["1i"][:], sinblk[:])
        nc.vector.tensor_mul(m2i[:], S["2i"][:], cosblk[:])
        nc.vector.tensor_add(x_i[:, 0:HCL], m1i[:, 0:HCL], m2i[:, 0:HCL])
        nc.vector.tensor_add(x_i[:, HCL:CL], m1i[:, HCL:CL], m2i[:, HCL:CL])

        pbu_io_pool.release()
        pbu_ie_pool.release()
        pbu_ro_pool.release()
        pbu_re_pool.release()
        po = tc.alloc_tile_pool(name="po", bufs=4, space="PSUM")

        # ---- projection slabs: out0 = Ctr@x_r (under the i chain),
        #      out1 = Cti@x_i (tail); host sums the slabs.
        # per chunk c the 2048 cols stay [evens 1024 | odds 1024] ----
        for slab, (Wt, x, outd) in enumerate(((Ctr, x_r, out0),
                                              (Cti, x_i, out1))):
            for c in range(FOLD):
                ps = slice(c * SLOC, (c + 1) * SLOC)
                st = stage.tile([128, CL], bt, tag="st")
                for hh in range(2):
                    pt = po.tile([128, 2 * JT], dt, tag="po")
                    for jh in range(2):
                        js = slice(hh * HCL + jh * JT,
                                   hh * HCL + (jh + 1) * JT)
                        nc.tensor.matmul(
                            pt[:, jh * JT:(jh + 1) * JT], Wt[ps, :],
                            x[ps, js], start=True, stop=True,
                            tile_position=(c * SLOC, 0),
                        )
                    # slab0 evacs ride the ACT engine under the i-chain;
                    # slab1 evacs land in the tail where DVE is free
                    if slab == 1 and hh == 0:
                        nc.vector.tensor_copy(st[:, hh * HCL:(hh + 1) * HCL],
                                              pt[:])
                    else:
                        nc.scalar.copy(st[:, hh * HCL:(hh + 1) * HCL], pt[:])
                nc.sync.dma_start(
                    out=outd[:, c * CL:(c + 1) * CL], in_=st[:])
        for p in (po, stage, big, cpool):
            p.release()
    if split_waits:
        _split_matmul_waits(nc, mybir)
    return nc


def _split_matmul_waits(nc, mybir):
    """Hardware instruction structs fit a limited number of embedded sync
    waits; move extra waits onto an inserted same-queue no-op."""
    caps = {"InstMatmult": 1}
    skip = {"InstNoOp", "InstAllEngineBarrier", "InstSync"}
    k = 0
    for bb in nc.main_func.blocks:
        insts = bb.instructions
        i = 0
        while i < len(insts):
            ins = insts[i]
            tn = type(ins).__name__
            if tn not in skip and ins.sync_info is not None:
                cap = caps.get(tn, 1)
                w = list(ins.sync_info.on_wait or [])
                if len(w) > cap:
                    for wj in w[:-cap]:
                        nop = mybir.InstNoOp(
                            name=f"I-mmdep-{k}",
                            engine=ins.engine,
                            ins=[],
                            outs=[],
                            sync_info=mybir.SyncInfo(
                                on_wait=[wj], on_update=[]
                            ),
                        )
                        k += 1
                        insts.insert(i, nop)
                        i += 1
                    ins.sync_info = mybir.SyncInfo(
                        on_wait=w[-cap:], on_update=ins.sync_info.on_update
                    )
            i += 1


def _eo_permute(a):
    """per 2048-col chunk: natural t' order -> [evens 1024 | odds 1024]."""
    r, n = a.shape
    nch = n // CL
    return np.ascontiguousarray(
        a.reshape(r, nch, CL // 2, 2).transpose(0, 1, 3, 2).reshape(r, n))


def _eo_unpermute(a):
    r, n = a.shape
    nch = n // CL
    return np.ascontiguousarray(
        a.reshape(r, nch, 2, CL // 2).transpose(0, 1, 3, 2).reshape(r, n))


def _host_prep(inputs):
    import ml_dtypes
    bf16 = ml_dtypes.bfloat16
    f32 = np.float32

    inp32 = np.asarray(inputs["input_sequence"], np.float32)
    inpT = _eo_permute(np.ascontiguousarray(inp32.T)).astype(bf16)
    A = np.maximum(np.asarray(inputs["A_diag_raw"], np.float64), 0.0)
    s = 1.0 / (1.0 + np.exp(-np.asarray(inputs["steps_raw"], np.float64)))
    Br = np.asarray(inputs["B_real"], np.float64)
    Bi = np.asarray(inputs["B_img"], np.float64)
    Cr = np.asarray(inputs["C_real"], np.float64)
    Ci = np.asarray(inputs["C_img"], np.float64)

    costh = 1.0 - s * s * A / 2.0
    sinth = np.sqrt(np.maximum(1.0 - costh * costh, 1e-300))
    theta = np.arctan2(sinth, costh)
    gamma = (s - s * s * A / 2.0) / sinth

    q = np.arange(NPART)
    Wm = ((q[:, None] % SLOC == q[None, :] % SLOC)
          & (q[:, None] // SLOC < q[None, :] // SLOC)).astype(f32)

    tvec = np.arange(CL, dtype=np.float64)
    twopi = 2.0 * np.pi

    in_maps = []
    for k in range(NCORES):
        sl = slice(k * SLOC, (k + 1) * SLOC)
        th = theta[sl]
        gm = gamma[sl]
        BCC = np.empty((128, 320), bf16)
        BCC[:, 0:SLOC] = (s[sl, None] * Br[sl]).T.astype(bf16)
        BCC[:, SLOC:2 * SLOC] = (s[sl, None] * Bi[sl]).T.astype(bf16)
        BCC[:, 2 * SLOC:2 * SLOC + H] = np.tile(
            Cr[:, sl].T, (FOLD, 1)).astype(bf16)
        BCC[:, 2 * SLOC + H:] = np.tile(
            -Ci[:, sl].T, (FOLD, 1)).astype(bf16)

        # tables per partition q = c*SLOC + s at global time t = c*CL + j
        ang = np.empty((NPART, CL), np.float64)
        for c in range(FOLD):
            ang[c * SLOC:(c + 1) * SLOC] = np.mod(
                (c * CL + tvec)[None, :] * th[:, None], twopi)
        sinA = np.sin(ang)
        cosA = np.cos(ang)
        gq = np.tile(gm, FOLD)[:, None]
        T1 = gq * cosA + sinA
        T2 = cosA - gq * sinA

        m = {"inpT": inpT, "BCC": BCC, "Wm": Wm}
        for nm, tb in (("T1blk", T1), ("T2blk", T2),
                       ("sinblk", sinA), ("cosblk", cosA)):
            m[nm] = _eo_permute(np.ascontiguousarray(tb)).astype(bf16)
        in_maps.append(m)
    return in_maps


LAST_RESULTS = None


def kernel(**inputs) -> np.ndarray:
    global LAST_RESULTS
    from concourse.bass_utils import run_bass_kernel_spmd

    if "nc" not in _CACHE:
        _CACHE["nc"] = _build_bass()
    nc = _CACHE["nc"]

    in_maps = _host_prep(inputs)
    res = run_bass_kernel_spmd(nc, in_maps, core_ids=list(range(NCORES)))
    LAST_RESULTS = res
    part = np.zeros((H, L), np.float32)
    for r in res.results:
        part += np.asarray(r["out0"], np.float32)
        part += np.asarray(r["out1"], np.float32)
    out = np.ascontiguousarray(_eo_unpermute(part).T)
    out += (np.asarray(inputs["input_sequence"], np.float32)
            * np.asarray(inputs["D"], np.float32)[None, :])
    return out


# revision 28
# speedup vs baseline: 1.0940x; 1.0149x over previous
"""LinOSS layer Trainium2 kernel, v4.2.

Math (same closed form as v3): the per-state 2x2 recurrence has eigenvalues
e^{+-i theta}; the scanned state collapses to rank-2 modulated prefix sums

    u     = s * Bu                     (s folded into B on host)
    E     = cumsum(T1 * u);  F = cumsum(T2 * u)     per complex part
    x_t   = sin(t th) * E_t + cos(t th) * F_t
    T1    = gamma*cos + sin;  T2 = cos - gamma*sin

Structure (keeps the 128 = 4 time-chunks x 32 states partition fold):
  - EVEN/ODD TIME SPLIT, done on the host: inpT columns are permuted per
    2048-chunk to [evens 1024 | odds 1024]; all tables pre-blocked to match.
    The DVE scan (2 cycles/col, no perf modes) then only runs over the
    1024 pair-sums P_j = y_{2j} + y_{2j+1}:
        S_{2j+1} = seed + cumsum(P)_j          (scan, half length)
        S_{2j}   = S_{2j+1} - y_{2j+1}         (aligned 2x tensor_tensor sub)
    halving the dominant scan cost.
  - u is evacuated from PSUM to bf16 SBUF by the ACT engine (4 separate
    e/o PSUM tiles so the even evacs start mid-load), so modulations and
    demodulations are all-bf16 full-width [128,2048] tensor_tensor ops in
    DVE 2x mode.
  - per-chunk carry sums come from ACT activation(Identity, accum_out=..)
    re-reading the pair-sums (off the DVE critical path); Wm matmul turns
    them into scan initial values (v3 mechanism).
  - DMA issue cost is ~0.6us PER dma_start on a HWDGE ring (measured): the
    issue stream is split across BOTH rings (sync + scalar), small tensors
    are packed into one transfer, and out-DMAs batch 2 evacs each.
  - projection/output as v3: two slabs out0/out1, host sums + un-permutes.
"""

import numpy as np

L, H, P = 8192, 128, 256
NCORES = 8
SLOC = P // NCORES          # states per core
FOLD = 4                    # time chunks folded into partitions
CL = L // FOLD              # 2048 free columns per partition row
HCL = CL // 2               # 1024 columns per even/odd half
NPART = FOLD * SLOC         # 128
JT = 512                    # matmul j-tile width

_CACHE: dict = {}


def _build_bass(split_waits=True):
    import concourse.bass as bass
    import concourse.mybir as mybir
    import concourse.tile as tile

    dt = mybir.dt.float32
    bt = mybir.dt.bfloat16
    Alu = mybir.AluOpType
    AF = mybir.ActivationFunctionType

    nc = bass.Bass(
        trn_type="TRN2",
        target_bir_lowering=False,
        debug=False,
        num_devices=NCORES,
    )

    inpT_d = nc.dram_tensor("inpT", [H, L], bt, kind="ExternalInput").ap()
    # packed: Bt [*,0:64] | Ctr [*,64:192] | Cti [*,192:320]
    BCC_d = nc.dram_tensor("BCC", [128, 320], bt, kind="ExternalInput").ap()
    Wm_d = nc.dram_tensor("Wm", [NPART, NPART], dt, kind="ExternalInput").ap()
    T1_d = nc.dram_tensor("T1blk", [NPART, CL], bt, kind="ExternalInput").ap()
    T2_d = nc.dram_tensor("T2blk", [NPART, CL], bt, kind="ExternalInput").ap()
    sin_d = nc.dram_tensor("sinblk", [NPART, CL], bt, kind="ExternalInput").ap()
    cos_d = nc.dram_tensor("cosblk", [NPART, CL], bt, kind="ExternalInput").ap()
    out0 = nc.dram_tensor("out0", [H, L], bt, kind="ExternalOutput").ap()
    out1 = nc.dram_tensor("out1", [H, L], bt, kind="ExternalOutput").ap()

    with tile.TileContext(nc) as tc:
        cpool = tc.alloc_tile_pool(name="const", bufs=1)
        big = tc.alloc_tile_pool(name="big", bufs=1)
        stage = tc.alloc_tile_pool(name="stage", bufs=4)
        pbu_re_pool = tc.alloc_tile_pool(name="pbu_re", bufs=1, space="PSUM")
        pbu_ro_pool = tc.alloc_tile_pool(name="pbu_ro", bufs=1, space="PSUM")
        pbu_ie_pool = tc.alloc_tile_pool(name="pbu_ie", bufs=1, space="PSUM")
        pbu_io_pool = tc.alloc_tile_pool(name="pbu_io", bufs=1, space="PSUM")

        # ---- loads; issue cost is ~0.6us per dma_start per ring, so the
        # stream is split across the two HWDGE rings (sync + scalar) ----
        # sync ring: the 8 inpT chunks (evens first), then T2/cos tables.
        inpT = big.tile([128, L], bt, tag="inpT")
        T1blk = big.tile([NPART, CL], bt, tag="T1blk")
        KB = L // 8  # 1024-col dma blocks; evens are dram blocks 0,2,4,6
        for k8 in (0, 2, 4, 6):
            nc.sync.dma_start(
                out=inpT[:, k8 * KB:(k8 + 1) * KB],
                in_=inpT_d[:, k8 * KB:(k8 + 1) * KB],
            )
        # T1 rides the sync ring between the even and odd input chunks:
        # it gates the first modulations, so it must land with the odds
        nc.sync.dma_start(out=T1blk[:], in_=T1_d)
        for k8 in (1, 3, 5, 7):
            nc.sync.dma_start(
                out=inpT[:, k8 * KB:(k8 + 1) * KB],
                in_=inpT_d[:, k8 * KB:(k8 + 1) * KB],
            )
        # tables are WAR-gated on inpT progress (gpsimd dummy reading an
        # inpT block + writing a dummy slot of the table tile) so the
        # fair-shared DMA bandwidth stays on the critical input chunks
        def gate(name, gate_k8, col=0):
            # the dummy write must overlap the region the gated DMA writes
            dummy = big.tile([NPART, CL], bt, tag=name)
            gd = cpool.tile([1, 8], dt, tag=f"g_{name}_{col}")
            nc.gpsimd.memset(dummy[0:1, col:col + 8], 0.0)
            nc.gpsimd.tensor_tensor(
                gd[:], dummy[0:1, col:col + 8],
                inpT[0:1, gate_k8 * KB:gate_k8 * KB + 8], Alu.add)
            real = big.tile([NPART, CL], bt, tag=name)
            return real

        T2blk = gate("T2blk", 6)       # after 4th even block
        nc.sync.dma_start(out=T2blk[:], in_=T2_d)
        # scalar ring: small packed tensors + T1/sin/cos tables.
        BCC = cpool.tile([128, 320], bt)
        nc.scalar.dma_start(out=BCC[:], in_=BCC_d)
        Wm = cpool.tile([NPART, NPART], dt)
        nc.scalar.dma_start(out=Wm[:], in_=Wm_d)
        sinblk = gate("sinblk", 3)     # after 2nd odd block
        cosblk = gate("cosblk", 3)
        Ctr = BCC[:, 2 * SLOC:2 * SLOC + H]
        Cti = BCC[:, 2 * SLOC + H:2 * SLOC + 2 * H]

        ones = cpool.tile([NPART, HCL], bt)
        nc.vector.memset(ones[:], 1.0)

        # ---- Bu matmuls; four separate psum tiles so the even-half evacs
        # can start while the odd chunks are still streaming ----
        pbu_re = pbu_re_pool.tile([NPART, HCL], dt, tag="bu_re")
        pbu_ro = pbu_ro_pool.tile([NPART, HCL], dt, tag="bu_ro")
        pbu_ie = pbu_ie_pool.tile([NPART, HCL], dt, tag="bu_ie")
        pbu_io = pbu_io_pool.tile([NPART, HCL], dt, tag="bu_io")
        bs_r = slice(0, SLOC)
        bs_i = slice(SLOC, 2 * SLOC)
        for half, (pr, pi) in ((0, (pbu_re, pbu_ie)), (1, (pbu_ro, pbu_io))):
            for c in range(FOLD):
                ps = slice(c * SLOC, (c + 1) * SLOC)
                for jh in range(2):
                    rhs = inpT[:, c * CL + half * HCL + jh * JT:
                               c * CL + half * HCL + (jh + 1) * JT]
                    js = slice(jh * JT, (jh + 1) * JT)
                    nc.tensor.matmul(pr[ps, js], BCC[:, bs_r], rhs,
                                     start=True, stop=True,
                                     tile_position=(0, c * SLOC))
                    nc.tensor.matmul(pi[ps, js], BCC[:, bs_i], rhs,
                                     start=True, stop=True,
                                     tile_position=(0, c * SLOC))

        # ---- ACT evac of u to bf16 SBUF; even halves first, then the
        # gated sin/cos dma issues, then odd halves (queue-order matters:
        # a gated dma_start blocks later entries of its issuing ring) ----
        u_r = big.tile([NPART, CL], bt, tag="u_r")
        u_i = big.tile([NPART, CL], bt, tag="u_i")
        nc.scalar.copy(u_r[:, 0:HCL], pbu_re[:])
        nc.scalar.copy(u_i[:, 0:HCL], pbu_ie[:])
        nc.scalar.dma_start(out=sinblk[:], in_=sin_d)
        nc.scalar.dma_start(out=cosblk[:], in_=cos_d)
        nc.scalar.copy(u_r[:, HCL:CL], pbu_ro[:])
        nc.scalar.copy(u_i[:, HCL:CL], pbu_io[:])

        # ---- modulations (tt 2x) + pair-sums + carries.  The even-half
        # mods run during the odd input DMAs; odd halves + pair sums chase
        # the odd evacs ----
        A = cpool.tile([NPART, 4], dt)
        offs = cpool.tile([NPART, 4], dt)
        Pdum = big.tile([NPART, HCL], bt, tag="Pdum")
        Y = {}
        Pq = {}
        quants = [("1r", T1blk, u_r, 0), ("2r", T2blk, u_r, 1),
                  ("1i", T1blk, u_i, 2), ("2i", T2blk, u_i, 3)]
        for q, T, u, ai in quants:
            Yt = big.tile([NPART, CL], bt, tag=f"Y{q}")
            nc.vector.tensor_mul(Yt[:, 0:HCL], T[:, 0:HCL], u[:, 0:HCL])
            Y[q] = Yt

        def modpair(qi):
            q, T, u, ai = quants[qi]
            Yt = Y[q]
            Pt = big.tile([NPART, HCL], bt, tag=f"P{q}")
            nc.vector.tensor_mul(Yt[:, HCL:CL], T[:, HCL:CL], u[:, HCL:CL])
            nc.vector.tensor_add(Pt[:], Yt[:, 0:HCL], Yt[:, HCL:CL])
            # chunk sums on ACT, off the DVE chain
            nc.scalar.activation(Pdum[:], Pt[:], AF.Identity,
                                 accum_out=A[:, ai:ai + 1])
            Pq[q] = Pt

        S = {}

        def scan(q, ai):
            St = big.tile([NPART, CL], bt, tag=f"S{q}")
            ini = (pbu_re[:, ai:ai + 1] if ai < 2
                   else offs[:, ai:ai + 1])
            bass.BassGpSimd.tensor_tensor_scan(
                nc.vector, St[:, HCL:CL], ones[:], Pq[q][:], ini,
                Alu.mult, Alu.add,
            )
            S[q] = St

        def fix(q):
            # S_even = S_odd - y_odd  (aligned, 2x)
            nc.vector.tensor_sub(S[q][:, 0:HCL], S[q][:, HCL:CL],
                                 Y[q][:, HCL:CL])

        # carry matmuls split per accumulator column: scan k waits only on
        # its own chunk-sum chain
        modpair(0)                      # Y1r odd, P1r
        nc.tensor.matmul(pbu_re[:, 0:1], Wm[:], A[:, 0:1],
                         start=True, stop=True)
        modpair(1)                      # Y2r odd, P2r
        nc.tensor.matmul(pbu_re[:, 1:2], Wm[:], A[:, 1:2],
                         start=True, stop=True)
        modpair(2)                      # Y1i, P1i  (fills DVE while carry lands)
        scan("1r", 0)
        modpair(3)                      # Y2i, P2i
        nc.tensor.matmul(pbu_ie[:, 0:2], Wm[:], A[:, 2:4],
                         start=True, stop=True)
        nc.scalar.copy(offs[:, 2:4], pbu_ie[:, 0:2])
        scan("2r", 1)
        fix("1r")
        fix("2r")
        m1r = big.tile([NPART, CL], bt, tag="m1r")
        m2r = big.tile([NPART, CL], bt, tag="m2r")
        nc.vector.tensor_mul(m1r[:], S["1r"][:], sinblk[:])
        nc.vector.tensor_mul(m2r[:], S["2r"][:], cosblk[:])
        scan("1i", 2)
        scan("2i", 3)
        fix("1i")
        fix("2i")
        m1i = big.tile([NPART, CL], bt, tag="m1i")
        m2i = big.tile([NPART, CL], bt, tag="m2i")
        x_i = big.tile([NPART, CL], bt, tag="x_i")
        nc.vector.tensor_mul(m1i[:], S["1i"][:], sinblk[:])
        nc.vector.tensor_mul(m2i[:], S["2i"][:], cosblk[:])
        nc.vector.tensor_add(x_i[:, 0:HCL], m1i[:, 0:HCL], m2i[:, 0:HCL])
        nc.vector.tensor_add(x_i[:, HCL:CL], m1i[:, HCL:CL], m2i[:, HCL:CL])

        pbu_io_pool.release()
        pbu_ie_pool.release()
        pbu_ro_pool.release()
        pbu_re_pool.release()
        po = tc.alloc_tile_pool(name="po", bufs=4, space="PSUM")

        # ---- projection slabs: out0 = Ctr@x_r (under the i chain),
        #      out1 = Cti@x_i (tail); host sums the slabs.
        # per chunk c the 2048 cols stay [evens 1024 | odds 1024] ----
        # slab0: out0 = Ctr@m1r + Ctr@m2r via PSUM accumulation (the x_r
        # add never runs on DVE; the doubled matmuls hide under the i-chain
        # where the PE is idle).  slab1 (tail, PE-bound): single x_i pass.
        for slab, (Wt, xs, outd) in enumerate(
                ((Ctr, (m1r, m2r), out0), (Cti, (x_i,), out1))):
            for c in range(FOLD):
                ps = slice(c * SLOC, (c + 1) * SLOC)
                st = stage.tile([128, CL], bt, tag="st")
                for hh in range(2):
                    pt = po.tile([128, 2 * JT], dt, tag="po")
                    for jh in range(2):
                        js = slice(hh * HCL + jh * JT,
                                   hh * HCL + (jh + 1) * JT)
                        for xi, x in enumerate(xs):
                            nc.tensor.matmul(
                                pt[:, jh * JT:(jh + 1) * JT], Wt[ps, :],
                                x[ps, js], start=(xi == 0),
                                stop=(xi == len(xs) - 1),
                                tile_position=(c * SLOC, 0),
                            )
                    # slab0 evacs ride the ACT engine under the i-chain;
                    # slab1 evacs land in the tail where DVE is free
                    if slab == 1 and hh == 0:
                        nc.vector.tensor_copy(st[:, hh * HCL:(hh + 1) * HCL],
                                              pt[:])
                    else:
                        nc.scalar.copy(st[:, hh * HCL:(hh + 1) * HCL], pt[:])
                nc.sync.dma_start(
                    out=outd[:, c * CL:(c + 1) * CL], in_=st[:])
        for p in (po, stage, big, cpool):
            p.release()
    if split_waits:
        _split_matmul_waits(nc, mybir)
    return nc


def _split_matmul_waits(nc, mybir):
    """Hardware instruction structs fit a limited number of embedded sync
    waits; move extra waits onto an inserted same-queue no-op."""
    caps = {"InstMatmult": 1}
    skip = {"InstNoOp", "InstAllEngineBarrier", "InstSync"}
    k = 0
    for bb in nc.main_func.blocks:
        insts = bb.instructions
        i = 0
        while i < len(insts):
            ins = insts[i]
            tn = type(ins).__name__
            if tn not in skip and ins.sync_info is not None:
                cap = caps.get(tn, 1)
                w = list(ins.sync_info.on_wait or [])
                if len(w) > cap:
                    for wj in w[:-cap]:
                        nop = mybir.InstNoOp(
                            name=f"I-mmdep-{k}",
                            engine=ins.engine,
                            ins=[],
                            outs=[],
                            sync_info=mybir.SyncInfo(
                                on_wait=[wj], on_update=[]
                            ),
                        )
                        k += 1
                        insts.insert(i, nop)
                        i += 1
                    ins.sync_info = mybir.SyncInfo(
                        on_wait=w[-cap:], on_update=ins.sync_info.on_update
                    )
            i += 1


def _eo_permute(a):
    """per 2048-col chunk: natural t' order -> [evens 1024 | odds 1024]."""
    r, n = a.shape
    nch = n // CL
    return np.ascontiguousarray(
        a.reshape(r, nch, CL // 2, 2).transpose(0, 1, 3, 2).reshape(r, n))


def _eo_unpermute(a):
    r, n = a.shape
    nch = n // CL
    return np.ascontiguousarray(
        a.reshape(r, nch, 2, CL // 2).transpose(0, 1, 3, 2).reshape(r, n))


def _host_prep(inputs):
    import ml_dtypes
    bf16 = ml_dtypes.bfloat16
    f32 = np.float32

    inp32 = np.asarray(inputs["input_sequence"], np.float32)
    inpT = _eo_permute(np.ascontiguousarray(inp32.T)).astype(bf16)
    A = np.maximum(np.asarray(inputs["A_diag_raw"], np.float64), 0.0)
    s = 1.0 / (1.0 + np.exp(-np.asarray(inputs["steps_raw"], np.float64)))
    Br = np.asarray(inputs["B_real"], np.float64)
    Bi = np.asarray(inputs["B_img"], np.float64)
    Cr = np.asarray(inputs["C_real"], np.float64)
    Ci = np.asarray(inputs["C_img"], np.float64)

    costh = 1.0 - s * s * A / 2.0
    sinth = np.sqrt(np.maximum(1.0 - costh * costh, 1e-300))
    theta = np.arctan2(sinth, costh)
    gamma = (s - s * s * A / 2.0) / sinth

    q = np.arange(NPART)
    Wm = ((q[:, None] % SLOC == q[None, :] % SLOC)
          & (q[:, None] // SLOC < q[None, :] // SLOC)).astype(f32)

    tvec = np.arange(CL, dtype=np.float64)
    twopi = 2.0 * np.pi

    in_maps = []
    for k in range(NCORES):
        sl = slice(k * SLOC, (k + 1) * SLOC)
        th = theta[sl]
        gm = gamma[sl]
        BCC = np.empty((128, 320), bf16)
        BCC[:, 0:SLOC] = (s[sl, None] * Br[sl]).T.astype(bf16)
        BCC[:, SLOC:2 * SLOC] = (s[sl, None] * Bi[sl]).T.astype(bf16)
        BCC[:, 2 * SLOC:2 * SLOC + H] = np.tile(
            Cr[:, sl].T, (FOLD, 1)).astype(bf16)
        BCC[:, 2 * SLOC + H:] = np.tile(
            -Ci[:, sl].T, (FOLD, 1)).astype(bf16)

        # tables per partition q = c*SLOC + s at global time t = c*CL + j
        ang = np.empty((NPART, CL), np.float64)
        for c in range(FOLD):
            ang[c * SLOC:(c + 1) * SLOC] = np.mod(
                (c * CL + tvec)[None, :] * th[:, None], twopi)
        sinA = np.sin(ang)
        cosA = np.cos(ang)
        gq = np.tile(gm, FOLD)[:, None]
        T1 = gq * cosA + sinA
        T2 = cosA - gq * sinA

        m = {"inpT": inpT, "BCC": BCC, "Wm": Wm}
        for nm, tb in (("T1blk", T1), ("T2blk", T2),
                       ("sinblk", sinA), ("cosblk", cosA)):
            m[nm] = _eo_permute(np.ascontiguousarray(tb)).astype(bf16)
        in_maps.append(m)
    return in_maps


LAST_RESULTS = None


def kernel(**inputs) -> np.ndarray:
    global LAST_RESULTS
    from concourse.bass_utils import run_bass_kernel_spmd

    if "nc" not in _CACHE:
        _CACHE["nc"] = _build_bass()
    nc = _CACHE["nc"]

    in_maps = _host_prep(inputs)
    res = run_bass_kernel_spmd(nc, in_maps, core_ids=list(range(NCORES)))
    LAST_RESULTS = res
    part = np.zeros((H, L), np.float32)
    for r in res.results:
        part += np.asarray(r["out0"], np.float32)
        part += np.asarray(r["out1"], np.float32)
    out = np.ascontiguousarray(_eo_unpermute(part).T)
    out += (np.asarray(inputs["input_sequence"], np.float32)
            * np.asarray(inputs["D"], np.float32)[None, :])
    return out
